# revision 3
# baseline (speedup 1.0000x reference)
import numpy as np
import jax
import jax.numpy as jnp

PI = 3.141592653589793
M_COEF = 1.01

F1 = np.array([1.0, 2.0, 1.0], dtype=np.float32) / 4.0
F2 = np.array([1, 2, 3, 4, 5, 4, 3, 2, 1], dtype=np.float32) / 25


def make_kernel(H, W, ncores, halo=7, xp=jnp):
    P = H // ncores
    N = P + 2 * halo
    XMAX = W - 1.001
    YMAX = H - 1.001
    f32 = np.float32

    def shift_x(A, k):
        if k == 0:
            return A
        z = xp.zeros((A.shape[0], abs(k)), A.dtype)
        if k > 0:
            return xp.concatenate([A[:, k:], z], axis=1)
        return xp.concatenate([z, A[:, :k]], axis=1)

    def shift_y(A, k):
        if k == 0:
            return A
        z = xp.zeros((abs(k), A.shape[1]), A.dtype)
        if k > 0:
            return xp.concatenate([A[k:], z], axis=0)
        return xp.concatenate([z, A[:k]], axis=0)

    def conv_h(A, f):
        r = len(f) // 2
        acc = None
        for i in range(len(f)):
            t = shift_x(A, i - r) * f32(f[i])
            acc = t if acc is None else acc + t
        return acc

    def conv_v(A, f):
        r = len(f) // 2
        acc = None
        for i in range(len(f)):
            t = shift_y(A, i - r) * f32(f[i])
            acc = t if acc is None else acc + t
        return acc

    def grad_x(A):
        left = xp.concatenate([A[:, :1], A[:, :-1]], axis=1)
        right = xp.concatenate([A[:, 1:], A[:, -1:]], axis=1)
        g = (right - left) * f32(0.5)
        first = A[:, 1:2] - A[:, 0:1]
        last = A[:, -1:] - A[:, -2:-1]
        return xp.concatenate([first, g[:, 1:-1], last], axis=1)

    def grad_y_global(A, gidx):
        up = xp.concatenate([A[:1], A[:-1]], axis=0)
        dn = xp.concatenate([A[1:], A[-1:]], axis=0)
        central = (dn - up) * f32(0.5)
        top = (gidx == 0)[:, None]
        bot = (gidx == H - 1)[:, None]
        return xp.where(top, dn - A, xp.where(bot, A - up, central))

    J_SETS = {2.0: (-1, 0, 1, 2), 1.0: (-1, 0, 1), -1.0: (-1, 0), -2.0: (-2, -1, 0)}
    K_SETS = {2.0: (-2, -1, 0, 1, 2), 1.0: (-1, 0, 1), -1.0: (-1, 0, 1), -2.0: (-2, -1, 0, 1, 2)}

    def core_fn(slab, r0):
        # slab: [N, W] f32 raw edge rows [r0-halo, r0+P+halo), zero outside global range
        gidx = r0 - halo + xp.arange(N, dtype=np.int32)
        e2d = conv_v(conv_h(slab, F1), F1)
        inb = ((gidx >= 0) & (gidx < H))[:, None]
        e2d = xp.where(inb, e2d, f32(0.0))
        a = conv_v(conv_h(e2d, F2), F2)
        oy = grad_y_global(a, gidx)
        ox = grad_x(a)
        oxx = grad_x(ox)
        oyy = grad_y_global(oy, gidx)
        oxy = grad_x(oy)
        ori = xp.remainder(xp.arctan(oyy * xp.sign(-oxy) / (oxx + f32(1e-5))), f32(PI))
        c = xp.cos(ori)
        s = xp.sin(ori)
        cc = c[halo:halo + P]
        ss = s[halo:halo + P]
        yg = (r0 + xp.arange(P, dtype=np.int32)).astype(np.float32)[:, None]
        xg = xp.arange(W, dtype=np.float32)[None, :]
        e2d_c = e2d[halo:halo + P]

        rowsh = {j: e2d[halo + j: halo + j + P] for j in range(-2, 3)}

        samples = []
        for d in (2.0, 1.0, -2.0, -1.0):
            df = f32(d)
            xc = xp.clip(df * cc + xg, f32(0.0), f32(XMAX))
            yc = xp.clip(df * ss + yg, f32(0.0), f32(YMAX))
            px = xc - xg
            py = yc - yg
            acc = None
            for j in J_SETS[d]:
                wy = xp.maximum(f32(1.0) - xp.abs(py - f32(j)), f32(0.0))
                inner = None
                for k in K_SETS[d]:
                    wx = xp.maximum(f32(1.0) - xp.abs(px - f32(k)), f32(0.0))
                    t = wx * shift_x(rowsh[j], k)
                    inner = t if inner is None else inner + t
                term = wy * inner
                acc = term if acc is None else acc + term
            samples.append(acc)

        e = e2d_c * f32(M_COEF)
        supp = (e < samples[0]) | (e < samples[1]) | (e < samples[2]) | (e < samples[3])
        keep = 1 - supp.astype(np.int32)
        out = e2d_c * keep.astype(e2d_c.dtype)
        return out, keep

    def build(edge2d):
        Epad = np.zeros((H + 2 * halo, W), np.float32)
        Epad[halo:halo + H] = edge2d
        slabs = np.stack([Epad[i * P: i * P + N] for i in range(ncores)])
        r0s = np.arange(ncores, dtype=np.int32) * P
        return slabs, r0s

    return core_fn, build


_H = _W = 4096
_NCORES = 8
_core_fn, _build = make_kernel(_H, _W, _NCORES)
_pmapped = None


def kernel(edge):
    global _pmapped
    edge = np.asarray(edge)
    e2 = edge.reshape(_H, _W).astype(np.float32, copy=False)
    slabs, r0s = _build(e2)
    if _pmapped is None:
        _pmapped = jax.pmap(_core_fn)
    o, k = _pmapped(slabs, r0s)
    out = np.asarray(o).reshape(1, 1, _H, _W).astype(np.float32, copy=False)
    keep = np.asarray(k).reshape(1, 1, _H, _W).astype(np.int32, copy=False)
    return out, keep


# revision 4
# speedup vs baseline: 10.5509x; 10.5509x over previous
import numpy as np
import jax
import jax.numpy as jnp
from concurrent.futures import ThreadPoolExecutor

PI = 3.141592653589793
M_COEF = 1.01

F1 = np.array([1.0, 2.0, 1.0], dtype=np.float32) / 4.0
F2 = np.array([1, 2, 3, 4, 5, 4, 3, 2, 1], dtype=np.float32) / 25


def make_kernel(H, W, ncores, halo=7, xp=jnp):
    P = H // ncores
    N = P + 2 * halo
    XMAX = W - 1.001
    YMAX = H - 1.001
    f32 = np.float32

    def shift_x(A, k):
        if k == 0:
            return A
        z = xp.zeros((A.shape[0], abs(k)), A.dtype)
        if k > 0:
            return xp.concatenate([A[:, k:], z], axis=1)
        return xp.concatenate([z, A[:, :k]], axis=1)

    def shift_y(A, k):
        if k == 0:
            return A
        z = xp.zeros((abs(k), A.shape[1]), A.dtype)
        if k > 0:
            return xp.concatenate([A[k:], z], axis=0)
        return xp.concatenate([z, A[:k]], axis=0)

    def conv_h(A, f):
        r = len(f) // 2
        acc = None
        for i in range(len(f)):
            t = shift_x(A, i - r) * f32(f[i])
            acc = t if acc is None else acc + t
        return acc

    def conv_v(A, f):
        r = len(f) // 2
        acc = None
        for i in range(len(f)):
            t = shift_y(A, i - r) * f32(f[i])
            acc = t if acc is None else acc + t
        return acc

    def grad_x(A):
        left = xp.concatenate([A[:, :1], A[:, :-1]], axis=1)
        right = xp.concatenate([A[:, 1:], A[:, -1:]], axis=1)
        g = (right - left) * f32(0.5)
        first = A[:, 1:2] - A[:, 0:1]
        last = A[:, -1:] - A[:, -2:-1]
        return xp.concatenate([first, g[:, 1:-1], last], axis=1)

    def grad_y_global(A, gidx):
        up = xp.concatenate([A[:1], A[:-1]], axis=0)
        dn = xp.concatenate([A[1:], A[-1:]], axis=0)
        central = (dn - up) * f32(0.5)
        top = (gidx == 0)[:, None]
        bot = (gidx == H - 1)[:, None]
        return xp.where(top, dn - A, xp.where(bot, A - up, central))

    J_SETS = {2.0: (-1, 0, 1, 2), 1.0: (-1, 0, 1), -1.0: (-1, 0), -2.0: (-2, -1, 0)}
    K_SETS = {2.0: (-2, -1, 0, 1, 2), 1.0: (-1, 0, 1), -1.0: (-1, 0, 1), -2.0: (-2, -1, 0, 1, 2)}
    PACKW = np.array([float(1 << i) for i in range(16)], dtype=np.float32)

    def core_fn(slab, r0):
        # slab: [N, W] f32 raw edge rows [r0-halo, r0+P+halo), zero outside global range
        gidx = r0 - halo + xp.arange(N, dtype=np.int32)
        e2d = conv_v(conv_h(slab, F1), F1)
        inb = ((gidx >= 0) & (gidx < H))[:, None]
        e2d = xp.where(inb, e2d, f32(0.0))
        a = conv_v(conv_h(e2d, F2), F2)
        oy = grad_y_global(a, gidx)
        ox = grad_x(a)
        oxx = grad_x(ox)
        oyy = grad_y_global(oy, gidx)
        oxy = grad_x(oy)
        # closed form: ori = remainder(arctan(t), pi); c=cos(ori), s=sin(ori)
        #  => s = |t|/sqrt(1+t^2) (>=0), c = sign-adjusted 1/sqrt(1+t^2)
        t = oyy * xp.sign(-oxy) / (oxx + f32(1e-5))
        t2 = t * t
        r = f32(1.0) / xp.sqrt(f32(1.0) + t2)
        c = xp.where(t < 0, -r, r)
        s = xp.where(xp.isinf(t2), f32(1.0), xp.abs(t) * r)
        cc = c[halo:halo + P]
        ss = s[halo:halo + P]
        yg = (r0 + xp.arange(P, dtype=np.int32)).astype(np.float32)[:, None]
        xg = xp.arange(W, dtype=np.float32)[None, :]
        e2d_c = e2d[halo:halo + P]

        rowsh = {j: e2d[halo + j: halo + j + P] for j in range(-2, 3)}

        samples = []
        for d in (2.0, 1.0, -2.0, -1.0):
            df = f32(d)
            xc = xp.clip(df * cc + xg, f32(0.0), f32(XMAX))
            yc = xp.clip(df * ss + yg, f32(0.0), f32(YMAX))
            px = xc - xg
            py = yc - yg
            acc = None
            for j in J_SETS[d]:
                wy = xp.maximum(f32(1.0) - xp.abs(py - f32(j)), f32(0.0))
                inner = None
                for k in K_SETS[d]:
                    wx = xp.maximum(f32(1.0) - xp.abs(px - f32(k)), f32(0.0))
                    tt = wx * shift_x(rowsh[j], k)
                    inner = tt if inner is None else inner + tt
                term = wy * inner
                acc = term if acc is None else acc + term
            samples.append(acc)

        e = e2d_c * f32(M_COEF)
        supp = (e < samples[0]) | (e < samples[1]) | (e < samples[2]) | (e < samples[3])
        keepf = f32(1.0) - supp.astype(np.float32)
        # bit-pack 16 keep bits per f32 (exact: sums < 2^16)
        packed = (keepf.reshape(P, W // 16, 16) * PACKW).sum(axis=-1)
        return packed

    def host_e2d(edge2d):
        # numpy mirror of device conv_v(conv_h(slab, F1), F1): same op order
        A = edge2d
        acc = None
        for i in range(3):
            k = i - 1
            if k == 0:
                t = A * F1[i]
            elif k > 0:
                t = np.concatenate([A[:, k:], np.zeros((A.shape[0], k), np.float32)], axis=1) * F1[i]
            else:
                t = np.concatenate([np.zeros((A.shape[0], -k), np.float32), A[:, :k]], axis=1) * F1[i]
            acc = t if acc is None else acc + t
        A = acc
        acc = None
        for i in range(3):
            k = i - 1
            if k == 0:
                t = A * F1[i]
            elif k > 0:
                t = np.concatenate([A[k:], np.zeros((k, A.shape[1]), np.float32)], axis=0) * F1[i]
            else:
                t = np.concatenate([np.zeros((-k, A.shape[1]), np.float32), A[:k]], axis=0) * F1[i]
            acc = t if acc is None else acc + t
        return acc

    def build(edge2d):
        Epad = np.zeros((H + 2 * halo, W), np.float32)
        Epad[halo:halo + H] = edge2d
        slabs = np.stack([Epad[i * P: i * P + N] for i in range(ncores)])
        r0s = np.arange(ncores, dtype=np.int32) * P
        return slabs, r0s

    def assemble(edge2d, packed):
        # packed: [ncores, P, W//16] f32 -> keep [H,W] int32, out = e2d*keep
        v = np.ascontiguousarray(packed).reshape(H, W // 16).astype(np.int32)
        bits = (v[:, :, None] >> np.arange(16, dtype=np.int32)) & 1
        keep = bits.reshape(H, W).astype(np.int32)
        e2d = host_e2d(edge2d)
        out = e2d * keep.astype(np.float32)
        return out[None, None], keep[None, None]

    return core_fn, build, assemble


_H = _W = 4096
_NCORES = 8
_core_fn, _build, _assemble = make_kernel(_H, _W, _NCORES)
_pmapped = None
_pool = None


def _upload(slabs, r0s):
    global _pool
    devs = jax.devices()[:_NCORES]
    if _pool is None:
        _pool = ThreadPoolExecutor(max_workers=_NCORES)
    futs = [_pool.submit(jax.device_put, slabs[i], devs[i]) for i in range(_NCORES)]
    slab_d = [f.result() for f in futs]
    r0_d = [jax.device_put(r0s[i], devs[i]) for i in range(_NCORES)]
    sd = jax.device_put_sharded(slab_d, devs) if hasattr(jax, "device_put_sharded") else slab_d
    rd = jax.device_put_sharded(r0_d, devs) if hasattr(jax, "device_put_sharded") else r0_d
    return sd, rd


def kernel(edge):
    global _pmapped
    edge = np.asarray(edge)
    e2 = np.ascontiguousarray(edge.reshape(_H, _W).astype(np.float32, copy=False))
    slabs, r0s = _build(e2)
    if _pmapped is None:
        _pmapped = jax.pmap(_core_fn)
    try:
        sd, rd = _upload(slabs, r0s)
        packed = np.asarray(_pmapped(sd, rd))
    except Exception:
        packed = np.asarray(_pmapped(slabs, r0s))
    return _assemble(e2, packed)


# revision 8
# speedup vs baseline: 10.6358x; 1.0080x over previous
import numpy as np
import jax
import jax.numpy as jnp
from concurrent.futures import ThreadPoolExecutor

PI = 3.141592653589793
M_COEF = 1.01

F1 = np.array([1.0, 2.0, 1.0], dtype=np.float32) / 4.0
F2 = np.array([1, 2, 3, 4, 5, 4, 3, 2, 1], dtype=np.float32) / 25


def make_kernel(H, W, ncores, halo=7, xp=jnp):
    P = H // ncores
    N = P + 2 * halo
    XMAX = W - 1.001
    YMAX = H - 1.001
    f32 = np.float32

    def shift_x(A, k):
        if k == 0:
            return A
        z = xp.zeros((A.shape[0], abs(k)), A.dtype)
        if k > 0:
            return xp.concatenate([A[:, k:], z], axis=1)
        return xp.concatenate([z, A[:, :k]], axis=1)

    def shift_y(A, k):
        if k == 0:
            return A
        z = xp.zeros((abs(k), A.shape[1]), A.dtype)
        if k > 0:
            return xp.concatenate([A[k:], z], axis=0)
        return xp.concatenate([z, A[:k]], axis=0)

    def conv_h(A, f):
        r = len(f) // 2
        acc = None
        for i in range(len(f)):
            t = shift_x(A, i - r) * f32(f[i])
            acc = t if acc is None else acc + t
        return acc

    def conv_v(A, f):
        r = len(f) // 2
        acc = None
        for i in range(len(f)):
            t = shift_y(A, i - r) * f32(f[i])
            acc = t if acc is None else acc + t
        return acc

    def grad_x(A):
        left = xp.concatenate([A[:, :1], A[:, :-1]], axis=1)
        right = xp.concatenate([A[:, 1:], A[:, -1:]], axis=1)
        g = (right - left) * f32(0.5)
        first = A[:, 1:2] - A[:, 0:1]
        last = A[:, -1:] - A[:, -2:-1]
        return xp.concatenate([first, g[:, 1:-1], last], axis=1)

    def grad_y_global(A, gidx):
        up = xp.concatenate([A[:1], A[:-1]], axis=0)
        dn = xp.concatenate([A[1:], A[-1:]], axis=0)
        central = (dn - up) * f32(0.5)
        top = (gidx == 0)[:, None]
        bot = (gidx == H - 1)[:, None]
        return xp.where(top, dn - A, xp.where(bot, A - up, central))

    J_SETS = {2.0: (-1, 0, 1, 2), 1.0: (-1, 0, 1), -1.0: (-1, 0), -2.0: (-2, -1, 0)}
    K_SETS = {2.0: (-2, -1, 0, 1, 2), 1.0: (-1, 0, 1), -1.0: (-1, 0, 1), -2.0: (-2, -1, 0, 1, 2)}
    PACKW = np.array([float(1 << i) for i in range(16)], dtype=np.float32)

    def core_fn(slab, r0):
        # slab: [N, W] f32 raw edge rows [r0-halo, r0+P+halo), zero outside global range
        gidx = r0 - halo + xp.arange(N, dtype=np.int32)
        e2d = conv_v(conv_h(slab, F1), F1)
        inb = ((gidx >= 0) & (gidx < H))[:, None]
        e2d = xp.where(inb, e2d, f32(0.0))
        a = conv_v(conv_h(e2d, F2), F2)
        oy = grad_y_global(a, gidx)
        ox = grad_x(a)
        oxx = grad_x(ox)
        oyy = grad_y_global(oy, gidx)
        oxy = grad_x(oy)
        # closed form: ori = remainder(arctan(t), pi); c=cos(ori), s=sin(ori)
        #  => s = |t|/sqrt(1+t^2) (>=0), c = sign-adjusted 1/sqrt(1+t^2)
        t = oyy * xp.sign(-oxy) / (oxx + f32(1e-5))
        t2 = t * t
        r = f32(1.0) / xp.sqrt(f32(1.0) + t2)
        c = xp.where(t < 0, -r, r)
        s = xp.where(xp.isinf(t2), f32(1.0), xp.abs(t) * r)
        cc = c[halo:halo + P]
        ss = s[halo:halo + P]
        yg = (r0 + xp.arange(P, dtype=np.int32)).astype(np.float32)[:, None]
        xg = xp.arange(W, dtype=np.float32)[None, :]
        e2d_c = e2d[halo:halo + P]

        rowsh = {j: e2d[halo + j: halo + j + P] for j in range(-2, 3)}

        samples = []
        for d in (2.0, 1.0, -2.0, -1.0):
            df = f32(d)
            xc = xp.clip(df * cc + xg, f32(0.0), f32(XMAX))
            yc = xp.clip(df * ss + yg, f32(0.0), f32(YMAX))
            px = xc - xg
            py = yc - yg
            acc = None
            for j in J_SETS[d]:
                wy = xp.maximum(f32(1.0) - xp.abs(py - f32(j)), f32(0.0))
                inner = None
                for k in K_SETS[d]:
                    wx = xp.maximum(f32(1.0) - xp.abs(px - f32(k)), f32(0.0))
                    tt = wx * shift_x(rowsh[j], k)
                    inner = tt if inner is None else inner + tt
                term = wy * inner
                acc = term if acc is None else acc + term
            samples.append(acc)

        e = e2d_c * f32(M_COEF)
        supp = (e < samples[0]) | (e < samples[1]) | (e < samples[2]) | (e < samples[3])
        keepf = f32(1.0) - supp.astype(np.float32)
        # bit-pack 16 keep bits per f32 (exact: sums < 2^16)
        packed = (keepf.reshape(P, W // 16, 16) * PACKW).sum(axis=-1)
        return packed

    def host_e2d(edge2d):
        # numpy mirror of device conv_v(conv_h(slab, F1), F1): same op order
        A = edge2d
        acc = None
        for i in range(3):
            k = i - 1
            if k == 0:
                t = A * F1[i]
            elif k > 0:
                t = np.concatenate([A[:, k:], np.zeros((A.shape[0], k), np.float32)], axis=1) * F1[i]
            else:
                t = np.concatenate([np.zeros((A.shape[0], -k), np.float32), A[:, :k]], axis=1) * F1[i]
            acc = t if acc is None else acc + t
        A = acc
        acc = None
        for i in range(3):
            k = i - 1
            if k == 0:
                t = A * F1[i]
            elif k > 0:
                t = np.concatenate([A[k:], np.zeros((k, A.shape[1]), np.float32)], axis=0) * F1[i]
            else:
                t = np.concatenate([np.zeros((-k, A.shape[1]), np.float32), A[:k]], axis=0) * F1[i]
            acc = t if acc is None else acc + t
        return acc

    def build(edge2d):
        Epad = np.zeros((H + 2 * halo, W), np.float32)
        Epad[halo:halo + H] = edge2d
        slabs = np.stack([Epad[i * P: i * P + N] for i in range(ncores)])
        r0s = np.arange(ncores, dtype=np.int32) * P
        return slabs, r0s

    def assemble(edge2d, packed, e2d=None):
        # packed: [ncores, P, W//16] f32 -> keep [H,W] int32, out = e2d*keep
        v16 = np.ascontiguousarray(packed).reshape(H, W // 16).astype(np.uint16)
        keep_u8 = np.unpackbits(v16.view(np.uint8), axis=1, bitorder="little")
        if e2d is None:
            e2d = host_e2d(edge2d)
        out = e2d * keep_u8
        keep = keep_u8.astype(np.int32)
        return out[None, None], keep[None, None]

    return core_fn, build, assemble, host_e2d


_H = _W = 4096
_NCORES = 8
_core_fn, _build, _assemble, _host_e2d = make_kernel(_H, _W, _NCORES)
_pmapped = None
_pool = None


def _upload(slabs, r0s):
    global _pool
    devs = jax.devices()[:_NCORES]
    if _pool is None:
        _pool = ThreadPoolExecutor(max_workers=_NCORES)
    futs = [_pool.submit(jax.device_put, slabs[i], devs[i]) for i in range(_NCORES)]
    slab_d = [f.result() for f in futs]
    r0_d = [jax.device_put(r0s[i], devs[i]) for i in range(_NCORES)]
    sd = jax.device_put_sharded(slab_d, devs) if hasattr(jax, "device_put_sharded") else slab_d
    rd = jax.device_put_sharded(r0_d, devs) if hasattr(jax, "device_put_sharded") else r0_d
    return sd, rd


_host_pool = ThreadPoolExecutor(max_workers=1)


def kernel(edge):
    global _pmapped
    edge = np.asarray(edge)
    e2 = np.ascontiguousarray(edge.reshape(_H, _W).astype(np.float32, copy=False))
    # host-side e2d conv overlaps with device upload/exec
    e2d_fut = _host_pool.submit(_host_e2d, e2)
    slabs, r0s = _build(e2)
    if _pmapped is None:
        _pmapped = jax.pmap(_core_fn)
    try:
        sd, rd = _upload(slabs, r0s)
        packed = np.asarray(_pmapped(sd, rd))
    except Exception:
        packed = np.asarray(_pmapped(slabs, r0s))
    return _assemble(e2, packed, e2d=e2d_fut.result())
